# revision 11
# baseline (speedup 1.0000x reference)
"""MD-RNN (4-direction 2D GRU) Trainium2 kernel, v2.

Sharding: direction x batch-half. Core c handles direction a = c % 4 for batch
half c // 4 (B=128 per core). All four directions are padded to a 29x29 scan
grid so every core runs the *same* program; the bwd directions' extra first
row/col are dummy cells whose patches and X_n stream are zeroed host-side,
which makes their hidden state exactly 0 (h = (1-z)*tanh(0) + z*0.5*(0+0)).

Per-core compute: 57 anti-diagonal wavefront steps over the 29x29 grid.
Layout: hidden-on-partition ("transposed"), h stored fp8e4 as [128, 2, cols]
(two 128-row chunks) so a single fp8 DoubleRow matmul contracts all 256 rows
at 0.5 cycles/row. Patches (16 pix + ones row + pad = 18 rows) also use
DoubleRow via a [9, 2, cols] layout. Recurrent weights are scaled by 16 on
host to dodge fp8e4 subnormals; the 1/16 is folded into the sigmoid's scale
operand and the n-gate's scalar_tensor_tensor scalar. The n-gate input
projection X_n (= patch @ Wx_n + b_n, which the reset gate does NOT multiply)
is precomputed on host in bf16 and streamed from DRAM, saving 2 of 6 patch
matmuls and 2 psum banks.

Gate math per chunk (fd = 4 cells x 128 batch = 512 cols, psum pairs span 2
banks so each activation op covers both 128-row chunks in one instruction):
  ACT   : r = sigmoid(psum/16), z = sigmoid(psum/16), n = tanh(t2)
  DVE   : t1 = (G_n/16) * r [psum], t2 = t1 + X_n, dt = 0.5*s - n,
          ht = e + n -> fp8
  GPSIMD: s = a + l (fp8 in), e = z * dt

Head: partial logits h_a^T @ W_out[a] per core (psum fp32), host sums the 4
direction partials, adds b_out, log_softmax.
"""

import numpy as np
import ml_dtypes

GRID = 4
N_IMG = 32
S = N_IMG - (GRID - 1)          # 29 scan positions per axis (padded, all dirs)
B_FULL = 256
N_CORES = 8
B = 128                          # batch per core (2 halves x 4 directions)
H = 256
OUT_DIM = 10
KP = 18                          # patch rows: 16 pixels + ones + pad
CELLS = 4                        # cells per chunk: 4*B = 512 = one psum bank
FD = CELLS * B                   # 512

RSCALE = 16.0                    # host-side weight scale (fp8 subnormal dodge)
REPEAT = 1                       # body repetitions (timing calibration only)

S_ENGINE = "gpsimd"              # engine for s = a + l (fp8 inputs)
E_ENGINE = "gpsimd"              # engine for e = z * dt
T2_ENGINE = "vector"             # engine for t2 = t1 + xn
HT_ENGINE = "vector"             # engine for ht = e + n (fp8 out)


def _diag_infos():
    """(ilo, ihi, cbase) per anti-diagonal of the padded 29x29 grid."""
    infos, base = [], 0
    for d in range(2 * S - 1):
        ilo = max(0, d - (S - 1))
        ihi = min(d, S - 1)
        infos.append((ilo, ihi, base))
        base += ihi - ilo + 1
    return infos, base


DIAG_INFOS, T_CELLS = _diag_infos()          # 57 diagonals, 841 cells


def _chunk_sizes(k):
    nch = (k + CELLS - 1) // CELLS
    lo = k // nch
    rem = k - lo * nch
    return [lo + 1] * rem + [lo] * (nch - rem)


def _scan_ij():
    """Scan-grid (i, j) of every cell in diag-major order."""
    ii, jj = [], []
    for d, (ilo, ihi, _) in enumerate(DIAG_INFOS):
        for i in range(ilo, ihi + 1):
            ii.append(i)
            jj.append(d - i)
    return np.asarray(ii), np.asarray(jj)


II, JJ = _scan_ij()

FWD = np.arange(S)
BWD_PAD = np.concatenate([[-1], np.arange(S - 2, -1, -1)])   # dummy row first
DIR_MAPS = [(FWD, FWD), (BWD_PAD, FWD), (FWD, BWD_PAD), (BWD_PAD, BWD_PAD)]

F8 = ml_dtypes.float8_e4m3
BF16 = ml_dtypes.bfloat16


def make_weight_maps(Wx, Uh, Uh2, b, W_out):
    """Per-direction device weight tensors (DoubleRow layouts, fp8/bf16)."""
    Wx = np.asarray(Wx, np.float32)
    Uh = np.asarray(Uh, np.float32)
    Uh2 = np.asarray(Uh2, np.float32)
    b = np.asarray(b, np.float32)
    W_out = np.asarray(W_out, np.float32)
    out = []
    for a in range(4):
        # uh: [128, 2, 2*768] fp8; [:, :, :768] = Uh pairs, rest Uh2
        u = (Uh[a] * RSCALE).reshape(2, 128, 768).transpose(1, 0, 2)
        u2 = (Uh2[a] * RSCALE).reshape(2, 128, 768).transpose(1, 0, 2)
        uh = np.concatenate([u, u2], axis=2).astype(F8)
        # wx: r,z gate columns only (512), rows [Wx(16); b(1); 0(1)] * 16
        wxa = np.zeros((KP, 512), np.float32)
        wxa[:16] = Wx[a][:, :512] * RSCALE
        wxa[16] = b[a][:512] * RSCALE
        wx = np.ascontiguousarray(
            wxa.reshape(2, 9, 512).transpose(1, 0, 2)).astype(F8)
        # wo: [128, 2, 10] bf16
        wo = np.ascontiguousarray(
            W_out[a * 256:(a + 1) * 256].reshape(2, 128, 10)
            .transpose(1, 0, 2)).astype(BF16)
        out.append({"uh": np.ascontiguousarray(uh), "wx": wx, "wo": wo})
    return out


def make_data_maps(x, Wx, b):
    """Per-(direction, half) patch stream pt [9,2,T*B] fp8 and X_n stream
    xn [128,2,T*B] bf16 (n-gate input projection + bias, zero on dummies)."""
    from numpy.lib.stride_tricks import sliding_window_view
    x = np.asarray(x, np.float32)
    Wx = np.asarray(Wx, np.float32)
    b = np.asarray(b, np.float32)
    maps = {}
    for half in range(2):
        xh = x[half * B:(half + 1) * B]
        w = sliding_window_view(xh, (GRID, GRID), axis=(1, 2))  # (B,29,29,4,4)
        for a in range(4):
            ymap, xmap = DIR_MAPS[a]
            iy = ymap[II]
            ix = xmap[JJ]
            valid = (iy >= 0) & (ix >= 0)
            iyc = np.where(valid, iy, 0)
            ixc = np.where(valid, ix, 0)
            pv = w[:, iyc, ixc].reshape(B, T_CELLS, 16).astype(np.float32)
            pv[:, ~valid] = 0.0
            # X_n = patch @ Wx_n + b_n (f32 host gemm), zero on dummy cells
            xn = pv.reshape(-1, 16) @ Wx[a][:, 512:] + b[a][512:]
            xn = xn.reshape(B, T_CELLS, 256)
            xn[:, ~valid] = 0.0
            # pt rows: [pv(16), ones, 0] -> [9, 2, T*B]
            p18 = np.zeros((KP, T_CELLS, B), np.float32)
            p18[:16] = pv.transpose(2, 1, 0)
            p18[16] = 1.0
            pt = np.ascontiguousarray(
                p18.reshape(2, 9, T_CELLS * B).swapaxes(0, 1)).astype(F8)
            xn_d = np.ascontiguousarray(
                xn.transpose(2, 1, 0).reshape(2, 128, T_CELLS * B)
                .swapaxes(0, 1)).astype(BF16)
            maps[(a, half)] = {"pt": pt, "xn": xn_d}
    return maps


def _build_nc():
    import concourse.bacc as bacc
    import concourse.mybir as mybir
    import concourse.tile as tile

    f32 = mybir.dt.float32
    bf16 = mybir.dt.bfloat16
    fp8 = mybir.dt.float8e4
    AF = mybir.ActivationFunctionType
    ALU = mybir.AluOpType
    DR = mybir.MatmulPerfMode.DoubleRow
    INV = 1.0 / RSCALE

    nc = bacc.Bacc("TRN2", target_bir_lowering=False, debug=False,
                   num_devices=N_CORES)
    pt_d = nc.dram_tensor("pt", [9, 2, T_CELLS * B], fp8, kind="ExternalInput")
    xn_d = nc.dram_tensor("xn", [128, 2, T_CELLS * B], bf16,
                          kind="ExternalInput")
    uh_d = nc.dram_tensor("uh", [128, 2, 2 * 768], fp8, kind="ExternalInput")
    wx_d = nc.dram_tensor("wx", [9, 2, 512], fp8, kind="ExternalInput")
    wo_d = nc.dram_tensor("wo", [128, 2, OUT_DIM], bf16, kind="ExternalInput")
    out_d = nc.dram_tensor("out", [128, OUT_DIM], f32, kind="ExternalOutput")

    eng = {"vector": None, "gpsimd": None}  # filled after nc exists

    with tile.TileContext(nc) as tc:
        from contextlib import ExitStack
        with ExitStack() as ctx:
            eng["vector"] = nc.vector
            eng["gpsimd"] = nc.gpsimd
            s_e = eng[S_ENGINE]
            e_e = eng[E_ENGINE]
            t2_e = eng[T2_ENGINE]
            ht_e = eng[HT_ENGINE]

            const = ctx.enter_context(tc.tile_pool(name="const", bufs=1))
            ptp = ctx.enter_context(tc.tile_pool(name="ptp", bufs=3))
            xnp = ctx.enter_context(tc.tile_pool(name="xnp", bufs=16))
            pp = ctx.enter_context(tc.tile_pool(name="pp", bufs=3,
                                                space="PSUM"))
            kap = ctx.enter_context(tc.tile_pool(name="kap", bufs=1,
                                                 space="PSUM"))
            hp = ctx.enter_context(tc.tile_pool(name="hp", bufs=3))
            ew = ctx.enter_context(tc.tile_pool(name="ew", bufs=3))
            hd = ctx.enter_context(tc.tile_pool(name="hd", bufs=1))

            uh_sb = const.tile([128, 2, 2 * 768], fp8, tag="uh")
            nc.sync.dma_start(out=uh_sb, in_=uh_d[:, :, :])
            wx_sb = const.tile([9, 2, 512], fp8, tag="wx")
            nc.sync.dma_start(out=wx_sb, in_=wx_d[:, :, :])
            wo_sb = const.tile([128, 2, OUT_DIM], bf16, tag="wo")
            nc.sync.dma_start(out=wo_sb, in_=wo_d[:, :, :])
            zero_h = const.tile([128, 2, 2 * B], fp8, tag="zeroh")
            nc.vector.memset(zero_h, 0.0)
            ka_ps = kap.tile([128, FD], f32, tag="ka")
            ka_w = const.tile([1, 1], bf16, tag="kaw")
            nc.vector.memset(ka_w, 1.0)

            def keepalive(dep_ap):
                # Tiny matmul hanging off a gate-math output: keeps the PE
                # activity window non-idle so HAM doesn't re-throttle the
                # clock during long dependency chains at small diagonals.
                nc.tensor.matmul(ka_ps[0:1, 0:1], ka_w, dep_ap,
                                 start=True, stop=True, skip_group_check=True)

            FLAT = "p a b -> p (a b)"
            nchunk = [0]
            pending = []   # deferred (dt, e, ht) clusters, one chunk behind

            def flush_pending():
                while pending:
                    pending.pop(0)()

            def emit_chunk(prev_t, s_a, cbase, c0, c1, ht, pt_t, small,
                           first, last):
                fd = (c1 - c0) * B
                full = fd == FD
                above = prev_t[:, :, (s_a + c0) * B:(s_a + c1) * B]
                left = prev_t[:, :, (s_a + 1 + c0) * B:(s_a + 1 + c1) * B]
                ptc = pt_t[:, :, c0 * B:c1 * B]

                xnt = xnp.tile([128, 2, FD], bf16, tag="xn")
                nc.sync.dma_start(
                    out=xnt[:, :, :fd],
                    in_=xn_d[:, :, (cbase + c0) * B:(cbase + c1) * B])

                # s = a + l early: independent of this chunk's matmuls
                st = ew.tile([128, 2, FD], bf16, tag="s")
                nc.gpsimd.tensor_add(st[:, :, :fd], above, left)

                pr = pp.tile([128, 2, FD], f32, tag="pp")
                pz = pp.tile([128, 2, FD], f32, tag="pp")
                pn = pp.tile([128, 2, FD], f32, tag="pp")
                for mc in range(2):        # r gates
                    po = pr[:, mc, :fd]
                    nc.tensor.matmul(po, uh_sb[:, :, mc * 128:(mc + 1) * 128],
                                     above, start=True, stop=False,
                                     perf_mode=DR)
                    nc.tensor.matmul(
                        po, uh_sb[:, :, 768 + mc * 128:768 + (mc + 1) * 128],
                        left, start=False, stop=False, perf_mode=DR)
                    nc.tensor.matmul(po, wx_sb[:, :, mc * 128:(mc + 1) * 128],
                                     ptc, start=False, stop=True, perf_mode=DR)
                for mc in range(2, 4):     # z gates
                    po = pz[:, mc - 2, :fd]
                    nc.tensor.matmul(po, uh_sb[:, :, mc * 128:(mc + 1) * 128],
                                     above, start=True, stop=False,
                                     perf_mode=DR)
                    nc.tensor.matmul(
                        po, uh_sb[:, :, 768 + mc * 128:768 + (mc + 1) * 128],
                        left, start=False, stop=False, perf_mode=DR)
                    nc.tensor.matmul(po, wx_sb[:, :, mc * 128:(mc + 1) * 128],
                                     ptc, start=False, stop=True, perf_mode=DR)
                for mc in range(4, 6):     # n gates (hidden only)
                    po = pn[:, mc - 4, :fd]
                    nc.tensor.matmul(po, uh_sb[:, :, mc * 128:(mc + 1) * 128],
                                     above, start=True, stop=False,
                                     perf_mode=DR)
                    nc.tensor.matmul(
                        po, uh_sb[:, :, 768 + mc * 128:768 + (mc + 1) * 128],
                        left, start=False, stop=True, perf_mode=DR)

                def fv(ap):
                    # full chunks use flat [128, 2*FD] APs (measurably faster
                    # on every engine); ragged chunks keep 3D strided slices
                    return ap.rearrange(FLAT) if full else ap[:, :, :fd]

                rt = ew.tile([128, 2, FD], bf16, tag="r")
                nc.scalar.activation(fv(rt), fv(pr), AF.Sigmoid, scale=INV)
                zt = ew.tile([128, 2, FD], bf16, tag="z")
                nc.scalar.activation(fv(zt), fv(pz), AF.Sigmoid, scale=INV)
                t1 = ew.tile([128, 2, FD], bf16, tag="t1")
                nc.vector.scalar_tensor_tensor(
                    fv(t1), fv(pn), INV, fv(rt), ALU.mult, ALU.mult)
                t2 = ew.tile([128, 2, FD], bf16, tag="t2")
                nc.vector.tensor_add(fv(t2), fv(t1), fv(xnt))
                nt = ew.tile([128, 2, FD], bf16, tag="n")
                nc.scalar.activation(fv(nt), fv(t2), AF.Tanh)
                # ht of the first chunks feeds the next diagonal's first
                # matmuls: route through DVE (fast) so the PE never waits on
                # the slower GpSimd queue at diagonal boundaries.
                if first:
                    ht_e = nc.vector
                else:
                    ht_e = nc.vector if nchunk[0] % 4 == 3 else nc.gpsimd
                nchunk[0] += 1

                def tail(st=st, nt=nt, zt=zt, ht=ht, c0=c0, c1=c1, fd=fd,
                         ht_e=ht_e, small=small, fv=fv):
                    dt_ = ew.tile([128, 2, FD], bf16, tag="d", name="d")
                    nc.vector.scalar_tensor_tensor(
                        fv(dt_), fv(st), 0.5, fv(nt), ALU.mult, ALU.subtract)
                    et = ew.tile([128, 2, FD], bf16, tag="e", name="e")
                    nc.vector.tensor_mul(fv(et), fv(zt), fv(dt_))
                    ht_e.tensor_add(ht[:, :, (1 + c0) * B:(1 + c1) * B],
                                    et[:, :, :fd], nt[:, :, :fd])
                    if small:
                        keepalive(nt[0:1, 0, 0:1])
                        keepalive(et[0:1, 0, 0:1])

                if small:
                    # short diagonals: no pipelining to gain, keep in order
                    flush_pending()
                    tail()
                else:
                    pending.append(tail)
                    if len(pending) > 1:
                        pending.pop(0)()

            for _rep in range(REPEAT):
                h_prev = None
                for d, (ilo, ihi, cbase) in enumerate(DIAG_INFOS):
                    k = ihi - ilo + 1
                    ht = hp.tile([128, 2, (S + 2) * B], fp8, tag="h")
                    nc.gpsimd.memset(ht[:, :, 0:B], 0.0)
                    nc.gpsimd.memset(ht[:, :, (k + 1) * B:(k + 2) * B], 0.0)
                    pt_t = ptp.tile([9, 2, S * B], fp8, tag="pt")
                    nc.sync.dma_start(
                        out=pt_t[:, :, :k * B],
                        in_=pt_d[:, :, cbase * B:(cbase + k) * B])
                    if d == 0:
                        prev_t, k_prev, ilo_prev = zero_h, 0, 0
                    else:
                        prev_t, k_prev, ilo_prev = h_prev
                    s_a = ilo - ilo_prev
                    assert 0 <= s_a and s_a + k <= k_prev + 2, d
                    small = k <= 8
                    c0 = 0
                    sizes = _chunk_sizes(k)
                    for ci, cs in enumerate(sizes):
                        emit_chunk(prev_t[:, :, :(k_prev + 2) * B], s_a,
                                   cbase, c0, c0 + cs,
                                   ht[:, :, :(k + 2) * B], pt_t, small,
                                   first=ci < 2, last=ci == len(sizes) - 1)
                        c0 += cs
                    h_prev = (ht, k, ilo)
                flush_pending()

            # head: partial logits = h_final^T @ W_out[a]  (host adds rest)
            ht, k, _ = h_prev
            assert k == 1
            hf = hd.tile([128, 2, B], bf16, tag="hf")
            nc.scalar.copy(hf, ht[:, :, B:2 * B])
            pl_t = pp.tile([128, 2, FD], f32, tag="pp")
            pl = pl_t[:, 0, :OUT_DIM]
            for kc in range(2):
                nc.tensor.matmul(pl, hf[:, kc, :], wo_sb[:, kc, :],
                                 start=(kc == 0), stop=(kc == 1))
            ot = hd.tile([128, OUT_DIM], f32, tag="ot")
            nc.scalar.copy(ot, pl)
            nc.sync.dma_start(out=out_d[:, :], in_=ot)

    nc.compile()
    return nc


_CACHE = {}


def get_nc():
    if "nc" not in _CACHE:
        _CACHE["nc"] = _build_nc()
    return _CACHE["nc"]


def make_in_maps(x, Wx, Uh, Uh2, b, W_out, b_out):
    wm = make_weight_maps(Wx, Uh, Uh2, b, W_out)
    dm = make_data_maps(x, Wx, b)
    in_maps = []
    for c in range(N_CORES):
        a, half = c % 4, c // 4
        m = dict(wm[a])
        m.update(dm[(a, half)])
        in_maps.append(m)
    return in_maps


def finish_host(partials, b_out):
    """partials: list of 8 [128, 10] partial-logit arrays -> full output."""
    b_out = np.asarray(b_out, np.float32)
    logits = np.zeros((B_FULL, OUT_DIM), np.float64)
    for c in range(N_CORES):
        a, half = c % 4, c // 4
        logits[half * B:(half + 1) * B] += np.asarray(partials[c], np.float64)
    logits += b_out
    mx = logits.max(axis=-1, keepdims=True)
    ex = np.exp(logits - mx)
    lse = np.log(ex.sum(axis=-1, keepdims=True)) + mx
    return (logits - lse).astype(np.float32)


def kernel(x, Wx, Uh, Uh2, b, W_out, b_out):
    from concourse.bass_utils import run_bass_kernel_spmd
    nc = get_nc()
    in_maps = make_in_maps(x, Wx, Uh, Uh2, b, W_out, b_out)
    res = run_bass_kernel_spmd(nc, in_maps, list(range(N_CORES)))
    return finish_host([res.results[c]["out"] for c in range(N_CORES)], b_out)


# revision 13
# speedup vs baseline: 1.0050x; 1.0050x over previous
"""MD-RNN (4-direction 2D GRU) Trainium2 kernel, v2.

Sharding: direction x batch-half. Core c handles direction a = c % 4 for batch
half c // 4 (B=128 per core). All four directions are padded to a 29x29 scan
grid so every core runs the *same* program; the bwd directions' extra first
row/col are dummy cells whose patches and X_n stream are zeroed host-side,
which makes their hidden state exactly 0 (h = (1-z)*tanh(0) + z*0.5*(0+0)).

Per-core compute: 57 anti-diagonal wavefront steps over the 29x29 grid.
Layout: hidden-on-partition ("transposed"), h stored fp8e4 as [128, 2, cols]
(two 128-row chunks) so a single fp8 DoubleRow matmul contracts all 256 rows
at 0.5 cycles/row. Patches (16 pix + ones row + pad = 18 rows) also use
DoubleRow via a [9, 2, cols] layout. Recurrent weights are scaled by 16 on
host to dodge fp8e4 subnormals; the 1/16 is folded into the sigmoid's scale
operand and the n-gate's scalar_tensor_tensor scalar. The n-gate input
projection X_n (= patch @ Wx_n + b_n, which the reset gate does NOT multiply)
is precomputed on host in bf16 and streamed from DRAM, saving 2 of 6 patch
matmuls and 2 psum banks.

Gate math per chunk (fd = 4 cells x 128 batch = 512 cols, psum pairs span 2
banks so each activation op covers both 128-row chunks in one instruction):
  ACT   : r = sigmoid(psum/16), z = sigmoid(psum/16), n = tanh(t2)
  DVE   : t1 = (G_n/16) * r [psum], t2 = t1 + X_n, dt = 0.5*s - n,
          ht = e + n -> fp8
  GPSIMD: s = a + l (fp8 in), e = z * dt

Head: partial logits h_a^T @ W_out[a] per core (psum fp32), host sums the 4
direction partials, adds b_out, log_softmax.
"""

import numpy as np
import ml_dtypes

GRID = 4
N_IMG = 32
S = N_IMG - (GRID - 1)          # 29 scan positions per axis (padded, all dirs)
B_FULL = 256
N_CORES = 8
B = 128                          # batch per core (2 halves x 4 directions)
H = 256
OUT_DIM = 10
KP = 18                          # patch rows: 16 pixels + ones + pad
CELLS = 4                        # cells per chunk: 4*B = 512 = one psum bank
FD = CELLS * B                   # 512

RSCALE = 16.0                    # host-side weight scale (fp8 subnormal dodge)
REPEAT = 1                       # body repetitions (timing calibration only)

S_ENGINE = "gpsimd"              # engine for s = a + l (fp8 inputs)
E_ENGINE = "gpsimd"              # engine for e = z * dt
T2_ENGINE = "vector"             # engine for t2 = t1 + xn
HT_ENGINE = "vector"             # engine for ht = e + n (fp8 out)


def _diag_infos():
    """(ilo, ihi, cbase) per anti-diagonal of the padded 29x29 grid."""
    infos, base = [], 0
    for d in range(2 * S - 1):
        ilo = max(0, d - (S - 1))
        ihi = min(d, S - 1)
        infos.append((ilo, ihi, base))
        base += ihi - ilo + 1
    return infos, base


DIAG_INFOS, T_CELLS = _diag_infos()          # 57 diagonals, 841 cells


def _chunk_sizes(k):
    nch = (k + CELLS - 1) // CELLS
    lo = k // nch
    rem = k - lo * nch
    return [lo + 1] * rem + [lo] * (nch - rem)


def _scan_ij():
    """Scan-grid (i, j) of every cell in diag-major order."""
    ii, jj = [], []
    for d, (ilo, ihi, _) in enumerate(DIAG_INFOS):
        for i in range(ilo, ihi + 1):
            ii.append(i)
            jj.append(d - i)
    return np.asarray(ii), np.asarray(jj)


II, JJ = _scan_ij()

FWD = np.arange(S)
BWD_PAD = np.concatenate([[-1], np.arange(S - 2, -1, -1)])   # dummy row first
DIR_MAPS = [(FWD, FWD), (BWD_PAD, FWD), (FWD, BWD_PAD), (BWD_PAD, BWD_PAD)]

F8 = ml_dtypes.float8_e4m3
BF16 = ml_dtypes.bfloat16


def make_weight_maps(Wx, Uh, Uh2, b, W_out):
    """Per-direction device weight tensors (DoubleRow layouts, fp8/bf16)."""
    Wx = np.asarray(Wx, np.float32)
    Uh = np.asarray(Uh, np.float32)
    Uh2 = np.asarray(Uh2, np.float32)
    b = np.asarray(b, np.float32)
    W_out = np.asarray(W_out, np.float32)
    out = []
    for a in range(4):
        # uh: [128, 2, 2*768] fp8; [:, :, :768] = Uh pairs, rest Uh2
        u = (Uh[a] * RSCALE).reshape(2, 128, 768).transpose(1, 0, 2)
        u2 = (Uh2[a] * RSCALE).reshape(2, 128, 768).transpose(1, 0, 2)
        uh = np.concatenate([u, u2], axis=2).astype(F8)
        # wx: r,z gate columns only (512), rows [Wx(16); b(1); 0(1)] * 16
        wxa = np.zeros((KP, 512), np.float32)
        wxa[:16] = Wx[a][:, :512] * RSCALE
        wxa[16] = b[a][:512] * RSCALE
        wx = np.ascontiguousarray(
            wxa.reshape(2, 9, 512).transpose(1, 0, 2)).astype(F8)
        # wo: [128, 2, 10] bf16
        wo = np.ascontiguousarray(
            W_out[a * 256:(a + 1) * 256].reshape(2, 128, 10)
            .transpose(1, 0, 2)).astype(BF16)
        out.append({"uh": np.ascontiguousarray(uh), "wx": wx, "wo": wo})
    return out


def make_data_maps(x, Wx, b):
    """Per-(direction, half) patch stream pt [9,2,T*B] fp8 and X_n stream
    xn [128,2,T*B] bf16 (n-gate input projection + bias, zero on dummies)."""
    from numpy.lib.stride_tricks import sliding_window_view
    x = np.asarray(x, np.float32)
    Wx = np.asarray(Wx, np.float32)
    b = np.asarray(b, np.float32)
    maps = {}
    for half in range(2):
        xh = x[half * B:(half + 1) * B]
        w = sliding_window_view(xh, (GRID, GRID), axis=(1, 2))  # (B,29,29,4,4)
        for a in range(4):
            ymap, xmap = DIR_MAPS[a]
            iy = ymap[II]
            ix = xmap[JJ]
            valid = (iy >= 0) & (ix >= 0)
            iyc = np.where(valid, iy, 0)
            ixc = np.where(valid, ix, 0)
            pv = w[:, iyc, ixc].reshape(B, T_CELLS, 16).astype(np.float32)
            pv[:, ~valid] = 0.0
            # X_n = patch @ Wx_n + b_n (f32 host gemm), zero on dummy cells
            xn = pv.reshape(-1, 16) @ Wx[a][:, 512:] + b[a][512:]
            xn = xn.reshape(B, T_CELLS, 256)
            xn[:, ~valid] = 0.0
            # pt rows: [pv(16), ones, 0] -> [9, 2, T*B]
            p18 = np.zeros((KP, T_CELLS, B), np.float32)
            p18[:16] = pv.transpose(2, 1, 0)
            p18[16] = 1.0
            pt = np.ascontiguousarray(
                p18.reshape(2, 9, T_CELLS * B).swapaxes(0, 1)).astype(F8)
            xn_d = np.ascontiguousarray(
                xn.transpose(2, 1, 0).reshape(2, 128, T_CELLS * B)
                .swapaxes(0, 1)).astype(BF16)
            maps[(a, half)] = {"pt": pt, "xn": xn_d}
    return maps


def _build_nc():
    import concourse.bacc as bacc
    import concourse.mybir as mybir
    import concourse.tile as tile

    f32 = mybir.dt.float32
    bf16 = mybir.dt.bfloat16
    fp8 = mybir.dt.float8e4
    AF = mybir.ActivationFunctionType
    ALU = mybir.AluOpType
    DR = mybir.MatmulPerfMode.DoubleRow
    INV = 1.0 / RSCALE

    nc = bacc.Bacc("TRN2", target_bir_lowering=False, debug=False,
                   num_devices=N_CORES)
    pt_d = nc.dram_tensor("pt", [9, 2, T_CELLS * B], fp8, kind="ExternalInput")
    xn_d = nc.dram_tensor("xn", [128, 2, T_CELLS * B], bf16,
                          kind="ExternalInput")
    uh_d = nc.dram_tensor("uh", [128, 2, 2 * 768], fp8, kind="ExternalInput")
    wx_d = nc.dram_tensor("wx", [9, 2, 512], fp8, kind="ExternalInput")
    wo_d = nc.dram_tensor("wo", [128, 2, OUT_DIM], bf16, kind="ExternalInput")
    out_d = nc.dram_tensor("out", [128, OUT_DIM], f32, kind="ExternalOutput")

    eng = {"vector": None, "gpsimd": None}  # filled after nc exists

    with tile.TileContext(nc) as tc:
        from contextlib import ExitStack
        with ExitStack() as ctx:
            eng["vector"] = nc.vector
            eng["gpsimd"] = nc.gpsimd
            s_e = eng[S_ENGINE]
            e_e = eng[E_ENGINE]
            t2_e = eng[T2_ENGINE]
            ht_e = eng[HT_ENGINE]

            const = ctx.enter_context(tc.tile_pool(name="const", bufs=1))
            ptp = ctx.enter_context(tc.tile_pool(name="ptp", bufs=3))
            xnp = ctx.enter_context(tc.tile_pool(name="xnp", bufs=16))
            pp = ctx.enter_context(tc.tile_pool(name="pp", bufs=3,
                                                space="PSUM"))
            kap = ctx.enter_context(tc.tile_pool(name="kap", bufs=1,
                                                 space="PSUM"))
            hp = ctx.enter_context(tc.tile_pool(name="hp", bufs=3))
            ew = ctx.enter_context(tc.tile_pool(name="ew", bufs=3))
            hd = ctx.enter_context(tc.tile_pool(name="hd", bufs=1))

            uh_sb = const.tile([128, 2, 2 * 768], fp8, tag="uh")
            nc.sync.dma_start(out=uh_sb, in_=uh_d[:, :, :])
            wx_sb = const.tile([9, 2, 512], fp8, tag="wx")
            nc.sync.dma_start(out=wx_sb, in_=wx_d[:, :, :])
            wo_sb = const.tile([128, 2, OUT_DIM], bf16, tag="wo")
            nc.sync.dma_start(out=wo_sb, in_=wo_d[:, :, :])
            zero_h = const.tile([128, 2, 2 * B], fp8, tag="zeroh")
            nc.vector.memset(zero_h, 0.0)
            ka_ps = kap.tile([128, FD], f32, tag="ka")
            ka_w = const.tile([1, 1], bf16, tag="kaw")
            nc.vector.memset(ka_w, 1.0)

            def keepalive(dep_ap):
                # Tiny matmul hanging off a gate-math output: keeps the PE
                # activity window non-idle so HAM doesn't re-throttle the
                # clock during long dependency chains at small diagonals.
                nc.tensor.matmul(ka_ps[0:1, 0:1], ka_w, dep_ap,
                                 start=True, stop=True, skip_group_check=True)

            FLAT = "p a b -> p (a b)"
            nchunk = [0]
            pending = []   # deferred (dt, e, ht) clusters, one chunk behind

            def flush_pending():
                while pending:
                    pending.pop(0)()

            def emit_chunk(prev_t, s_a, cbase, c0, c1, ht, pt_t, small,
                           first, last):
                fd = (c1 - c0) * B
                full = fd == FD
                above = prev_t[:, :, (s_a + c0) * B:(s_a + c1) * B]
                left = prev_t[:, :, (s_a + 1 + c0) * B:(s_a + 1 + c1) * B]
                ptc = pt_t[:, :, c0 * B:c1 * B]

                xnt = xnp.tile([128, 2, FD], bf16, tag="xn")
                nc.sync.dma_start(
                    out=xnt[:, :, :fd],
                    in_=xn_d[:, :, (cbase + c0) * B:(cbase + c1) * B])

                # s = a + l early: independent of this chunk's matmuls
                st = ew.tile([128, 2, FD], bf16, tag="s")
                nc.gpsimd.tensor_add(st[:, :, :fd], above, left)

                pr = pp.tile([128, 2, FD], f32, tag="pp")
                pz = pp.tile([128, 2, FD], f32, tag="pp")
                pn = pp.tile([128, 2, FD], f32, tag="pp")
                for mc in range(2):        # r gates
                    po = pr[:, mc, :fd]
                    nc.tensor.matmul(po, uh_sb[:, :, mc * 128:(mc + 1) * 128],
                                     above, start=True, stop=False,
                                     perf_mode=DR)
                    nc.tensor.matmul(
                        po, uh_sb[:, :, 768 + mc * 128:768 + (mc + 1) * 128],
                        left, start=False, stop=False, perf_mode=DR)
                    nc.tensor.matmul(po, wx_sb[:, :, mc * 128:(mc + 1) * 128],
                                     ptc, start=False, stop=True, perf_mode=DR)
                for mc in range(2, 4):     # z gates
                    po = pz[:, mc - 2, :fd]
                    nc.tensor.matmul(po, uh_sb[:, :, mc * 128:(mc + 1) * 128],
                                     above, start=True, stop=False,
                                     perf_mode=DR)
                    nc.tensor.matmul(
                        po, uh_sb[:, :, 768 + mc * 128:768 + (mc + 1) * 128],
                        left, start=False, stop=False, perf_mode=DR)
                    nc.tensor.matmul(po, wx_sb[:, :, mc * 128:(mc + 1) * 128],
                                     ptc, start=False, stop=True, perf_mode=DR)
                for mc in range(4, 6):     # n gates (hidden only)
                    po = pn[:, mc - 4, :fd]
                    nc.tensor.matmul(po, uh_sb[:, :, mc * 128:(mc + 1) * 128],
                                     above, start=True, stop=False,
                                     perf_mode=DR)
                    nc.tensor.matmul(
                        po, uh_sb[:, :, 768 + mc * 128:768 + (mc + 1) * 128],
                        left, start=False, stop=True, perf_mode=DR)

                def fv(ap):
                    # full chunks use flat [128, 2*FD] APs (measurably faster
                    # on every engine); ragged chunks keep 3D strided slices
                    return ap.rearrange(FLAT) if full else ap[:, :, :fd]

                rt = ew.tile([128, 2, FD], bf16, tag="r")
                nc.scalar.activation(fv(rt), fv(pr), AF.Sigmoid, scale=INV)
                zt = ew.tile([128, 2, FD], bf16, tag="z")
                nc.scalar.activation(fv(zt), fv(pz), AF.Sigmoid, scale=INV)
                t1 = ew.tile([128, 2, FD], bf16, tag="t1")
                nc.vector.scalar_tensor_tensor(
                    fv(t1), fv(pn), INV, fv(rt), ALU.mult, ALU.mult)
                t2 = ew.tile([128, 2, FD], bf16, tag="t2")
                nc.vector.tensor_add(fv(t2), fv(t1), fv(xnt))
                nt = ew.tile([128, 2, FD], bf16, tag="n")
                nc.scalar.activation(fv(nt), fv(t2), AF.Tanh)
                # ht of the first chunks feeds the next diagonal's first
                # matmuls: route through DVE (fast) so the PE never waits on
                # the slower GpSimd queue at diagonal boundaries.
                if first:
                    ht_e = nc.vector
                else:
                    ht_e = nc.vector if nchunk[0] % 4 == 3 else nc.gpsimd
                nchunk[0] += 1

                def tail(st=st, nt=nt, zt=zt, ht=ht, c0=c0, c1=c1, fd=fd,
                         ht_e=ht_e, small=small, fv=fv):
                    dt_ = ew.tile([128, 2, FD], bf16, tag="d", name="d")
                    nc.vector.scalar_tensor_tensor(
                        fv(dt_), fv(st), 0.5, fv(nt), ALU.mult, ALU.subtract)
                    et = ew.tile([128, 2, FD], bf16, tag="e", name="e")
                    nc.vector.tensor_mul(fv(et), fv(zt), fv(dt_))
                    ht_e.tensor_add(ht[:, :, (1 + c0) * B:(1 + c1) * B],
                                    et[:, :, :fd], nt[:, :, :fd])
                    if small:
                        keepalive(nt[0:1, 0, 0:1])
                        keepalive(et[0:1, 0, 0:1])

                if small:
                    # short diagonals: no pipelining to gain, keep in order
                    flush_pending()
                    tail()
                else:
                    pending.append(tail)
                    if len(pending) > 1:
                        pending.pop(0)()

            for _rep in range(REPEAT):
                h_prev = None
                for d, (ilo, ihi, cbase) in enumerate(DIAG_INFOS):
                    k = ihi - ilo + 1
                    ht = hp.tile([128, 2, (S + 2) * B], fp8, tag="h")
                    nc.gpsimd.memset(ht[:, :, 0:B], 0.0)
                    nc.gpsimd.memset(ht[:, :, (k + 1) * B:(k + 2) * B], 0.0)
                    pt_t = ptp.tile([9, 2, S * B], fp8, tag="pt")
                    nc.sync.dma_start(
                        out=pt_t[:, :, :k * B],
                        in_=pt_d[:, :, cbase * B:(cbase + k) * B])
                    if d == 0:
                        prev_t, k_prev, ilo_prev = zero_h, 0, 0
                    else:
                        prev_t, k_prev, ilo_prev = h_prev
                    s_a = ilo - ilo_prev
                    assert 0 <= s_a and s_a + k <= k_prev + 2, d
                    small = k <= 8
                    c0 = 0
                    sizes = _chunk_sizes(k)
                    for ci, cs in enumerate(sizes):
                        emit_chunk(prev_t[:, :, :(k_prev + 2) * B], s_a,
                                   cbase, c0, c0 + cs,
                                   ht[:, :, :(k + 2) * B], pt_t, small,
                                   first=ci < 2, last=ci == len(sizes) - 1)
                        c0 += cs
                    h_prev = (ht, k, ilo)
                flush_pending()

            # head: partial logits = h_final^T @ W_out[a]  (host adds rest)
            ht, k, _ = h_prev
            assert k == 1
            hf = hd.tile([128, 2, B], bf16, tag="hf")
            nc.scalar.copy(hf, ht[:, :, B:2 * B])
            pl_t = pp.tile([128, 2, FD], f32, tag="pp")
            pl = pl_t[:, 0, :OUT_DIM]
            for kc in range(2):
                nc.tensor.matmul(pl, hf[:, kc, :], wo_sb[:, kc, :],
                                 start=(kc == 0), stop=(kc == 1))
            ot = hd.tile([128, OUT_DIM], f32, tag="ot")
            nc.scalar.copy(ot, pl)
            nc.sync.dma_start(out=out_d[:, :], in_=ot)

    nc.compile()
    return nc


_CACHE = {}


def get_nc():
    if "nc" not in _CACHE:
        _CACHE["nc"] = _build_nc()
    return _CACHE["nc"]


def make_in_maps(x, Wx, Uh, Uh2, b, W_out, b_out):
    wm = make_weight_maps(Wx, Uh, Uh2, b, W_out)
    dm = make_data_maps(x, Wx, b)
    in_maps = []
    for c in range(N_CORES):
        a, half = c % 4, c // 4
        m = dict(wm[a])
        m.update(dm[(a, half)])
        in_maps.append(m)
    return in_maps


def finish_host(partials, b_out):
    """partials: list of 8 [128, 10] partial-logit arrays -> full output."""
    b_out = np.asarray(b_out, np.float32)
    logits = np.zeros((B_FULL, OUT_DIM), np.float64)
    for c in range(N_CORES):
        a, half = c % 4, c // 4
        logits[half * B:(half + 1) * B] += np.asarray(partials[c], np.float64)
    logits += b_out
    mx = logits.max(axis=-1, keepdims=True)
    ex = np.exp(logits - mx)
    lse = np.log(ex.sum(axis=-1, keepdims=True)) + mx
    return (logits - lse).astype(np.float32)


def kernel(x, Wx, Uh, Uh2, b, W_out, b_out):
    from concourse.bass_utils import run_bass_kernel_spmd
    nc = get_nc()
    in_maps = make_in_maps(x, Wx, Uh, Uh2, b, W_out, b_out)
    res = run_bass_kernel_spmd(nc, in_maps, list(range(N_CORES)))
    return finish_host([res.results[c]["out"] for c in range(N_CORES)], b_out)
